# revision 1
# baseline (speedup 1.0000x reference)
"""Trainium2 Bass kernel for Tacotron-style attention (nn_Attention_12704513261859).

Computation (per batch b):
    e[t, h]   = tanh( cbhg[b] @ W1[:D] + rnn[b] @ W1[D:] + b1 )[t, h]
    en[t]     = relu( e[t, :] @ W2 + b2 )
    aw        = softmax(en over t)
    context   = aw @ cbhg[b]            -> [1, D]
    out2      = rnn reshaped [B, 1, D]  (pure reshape, done host-side)

Sharding: pure data-parallel over batch B=32 across 8 NeuronCores
(4 batches per core); tiny Dense weights replicated.

fp32 matmuls on the TRN2 PE cost ~4x bf16 (hi/lo double pass at half
stream rate), so the matmul datapaths run in bf16 with fp32 PSUM
accumulation:
  - cast X tiles to bf16 once (ScalarE/VectorE split)
  - PE-transpose the bf16 tiles (FWL weight load) for the e-matmul,
    which must contract d on partitions
  - e-matmul / energies / context all bf16 inputs, fp32 accumulate
  - softmax denominator computed from the same bf16-rounded weights
exp(relu(x + b2)) is computed exactly as max(exp(x + b2), 1).

Pipelining: weight loads ride the gpsimd (SWDGE) ring in parallel with
the x stream on the sync (HWDGE) ring; per-batch work is emitted with a
one-stage skew and split by T-halves so the tail after the last DMA is
short and the PE stays dense (HAM stays at the 2.4 GHz clock).
"""

import os
import numpy as np

B, T, D, H = 32, 1024, 512, 10
NCORES = 8
BP = B // NCORES          # batches per core = 4
NT = T // 128             # 8 t-chunks
ND = D // 128             # 4 d-chunks

_CACHE = {}


def _build_nc():
    from contextlib import ExitStack

    import concourse.bass as bass
    import concourse.mybir as mybir
    import concourse.tile as tile
    from concourse import bacc
    from concourse.masks import make_identity

    f32 = mybir.dt.float32
    bf16 = mybir.dt.bfloat16

    nc = bacc.Bacc("TRN2", target_bir_lowering=False, debug=False, num_devices=NCORES)

    x_d = nc.dram_tensor("x", [BP, T, D], f32, kind="ExternalInput")
    rnn_d = nc.dram_tensor("rnn", [BP, D], f32, kind="ExternalInput")
    w1_d = nc.dram_tensor("w1", [2 * D, H], f32, kind="ExternalInput")
    b1_d = nc.dram_tensor("b1", [H], f32, kind="ExternalInput")
    w2_d = nc.dram_tensor("w2", [H, 1], f32, kind="ExternalInput")
    b2_d = nc.dram_tensor("b2", [1], f32, kind="ExternalInput")
    rb_d = nc.dram_tensor("rb", [H, BP], f32, kind="ExternalInput")
    out_d = nc.dram_tensor("ctx_out", [BP, D], f32, kind="ExternalOutput")

    with tile.TileContext(nc) as tc, ExitStack() as ctx:
        consts = ctx.enter_context(tc.tile_pool(name="consts", bufs=1))
        xp = ctx.enter_context(tc.tile_pool(name="xp", bufs=1))
        xbp = ctx.enter_context(tc.tile_pool(name="xbp", bufs=BP))
        xtp = ctx.enter_context(tc.tile_pool(name="xtp", bufs=BP))
        ep = ctx.enter_context(tc.tile_pool(name="ep", bufs=2))
        sp = ctx.enter_context(tc.tile_pool(name="sp", bufs=2))
        op = ctx.enter_context(tc.tile_pool(name="op", bufs=2))
        ptr = ctx.enter_context(tc.tile_pool(name="ptr", bufs=2, space="PSUM"))
        pwm = ctx.enter_context(tc.tile_pool(name="pwm", bufs=1, space="PSUM"))
        pe_ps = ctx.enter_context(tc.tile_pool(name="pe_ps", bufs=2, space="PSUM"))
        psm = ctx.enter_context(tc.tile_pool(name="psm", bufs=2, space="PSUM"))
        pcx = ctx.enter_context(tc.tile_pool(name="pcx", bufs=1, space="PSUM"))

        # ---- PE pre-warm: dummy transposes while waiting for data keep the
        # HAM activity window busy so real matmuls run at 2.4 GHz ----
        wset = consts.tile([128, 128], bf16)
        nc.vector.memset(wset, 0.0)
        warm_ps = pwm.tile([128, 128], f32, tag="warm", name="warm_ps")
        for _ in range(24):
            nc.tensor.matmul(warm_ps, wset, wset, start=True, stop=True)

        # ---- constants / weights ----
        # identity on gpsimd; the x casting-DMAs follow on the same SWDGE ring
        ident_f = consts.tile([128, 128], f32)
        make_identity(nc, ident_f)

        # ---- bulk x loads: HBM fp32 -> SBUF bf16 via casting SWDGE DMA
        # (fp32 X never lands in SBUF). Batch 0's first half goes fp32 over
        # the otherwise-idle sync ring + engine casts so its transposes can
        # start right as the PE pre-warm ends ----
        xc_tiles = []
        x0h = xp.tile([128, 4, D], f32, tag="x0h", name="x0h")
        src0 = x_d[0].rearrange("(i p) d -> p i d", p=128)
        nc.sync.dma_start(out=x0h[:, 0:2, :], in_=src0[:, 0:2, :])
        nc.sync.dma_start(out=x0h[:, 2:4, :], in_=src0[:, 2:4, :])
        for b in range(BP):
            xc = xbp.tile([128, NT, D], bf16, tag="xc", name=f"xc{b}")
            src = x_d[b].rearrange("(i p) d -> p i d", p=128)
            q0 = 4 if b == 0 else 0
            for q in range(q0, NT, 4):
                nc.gpsimd.dma_start(
                    out=xc[:, q : q + 4, :], in_=src[:, q : q + 4, :]
                )
            xc_tiles.append(xc)

        # small weights after x0 on the sync ring
        rb = consts.tile([H, BP], f32)
        nc.sync.dma_start(out=rb, in_=rb_d[:, :])
        w1ab = consts.tile([128, ND, H], f32)
        nc.sync.dma_start(
            out=w1ab, in_=w1_d[0:D, :].rearrange("(c p) h -> p c h", p=128)
        )
        w1a = w1ab
        w2s = consts.tile([H, 1], f32)
        nc.sync.dma_start(out=w2s, in_=w2_d[:, :])

        b2s = consts.tile([128, 1], f32)
        b2_ap = b2_d[:]
        b2_bcast = bass.AP(
            tensor=b2_ap.tensor, offset=b2_ap.offset, ap=[[0, 128]] + list(b2_ap.ap)
        )
        nc.sync.dma_start(out=b2s, in_=b2_bcast)

        # early DVE ops (before any dependency on PE results)
        ident = consts.tile([128, 128], bf16)
        nc.vector.tensor_copy(ident, ident_f)
        ones = consts.tile([128, 1], bf16)
        nc.vector.memset(ones, 1.0)

        # batch-0 first-half casts (VectorE/ScalarE)
        v0s = x0h.rearrange("p i d -> p (i d)")
        v0d = xc_tiles[0].rearrange("p i d -> p (i d)")
        nc.vector.tensor_copy(v0d[:, 0:1024], v0s[:, 0:1024])
        nc.scalar.copy(v0d[:, 1024:2048], v0s[:, 1024:2048])

        xt_tiles = [None] * BP   # bf16 transposed
        e_tiles = [None] * BP
        expE_tiles = [None] * BP

        def transposes(b, halves=((0, 1),)):
            # per d-chunk j: transposes grouped into one psum tile; batch 0
            # runs T-halves separately so its first half (delivered early via
            # the sync ring) is consumed before the SWDGE half lands
            xt_tiles[b] = xtp.tile([128, ND, T], bf16, tag="xt", name=f"xt{b}")
            xt = xt_tiles[b]
            for hs in halves:
                for j in range(ND):
                    w = 512 * len(hs)
                    tps = ptr.tile(
                        [128, 1024], bf16, tag="tps", name=f"tps{b}_{hs[0]}_{j}"
                    )
                    for k, h in enumerate(hs):
                        for q in range(4):
                            i = h * 4 + q
                            nc.tensor.transpose(
                                tps[:, (k * 4 + q) * 128 : (k * 4 + q + 1) * 128],
                                xc_tiles[b][:, i, j * 128 : (j + 1) * 128],
                                ident,
                            )
                    dst = xt[:, j, hs[0] * 512 : hs[0] * 512 + w]
                    if j == 3:
                        nc.scalar.copy(dst, tps[:, 0:w])
                    else:
                        nc.vector.tensor_copy(dst, tps[:, 0:w])

        def e_mm(b):
            # both T-half chains interleaved: consecutive matmuls accumulate
            # into different PSUM banks, so fills overlap the drains
            e_tiles[b] = ep.tile([H, T], bf16, tag="e", name=f"e{b}")
            e_sb = e_tiles[b]
            eps0 = pe_ps.tile([H, 512], f32, tag="eps", name=f"eps{b}_0")
            eps1 = pe_ps.tile([H, 512], f32, tag="eps", name=f"eps{b}_1")
            for j in range(ND):
                nc.tensor.matmul(
                    eps0, w1a_b[:, j, :], xt_tiles[b][:, j, 0:512],
                    start=(j == 0), stop=(j == ND - 1),
                )
                nc.tensor.matmul(
                    eps1, w1a_b[:, j, :], xt_tiles[b][:, j, 512:1024],
                    start=(j == 0), stop=(j == ND - 1),
                )
            for n, eps in ((0, eps0), (1, eps1)):
                nc.scalar.activation(
                    e_sb[:, n * 512 : (n + 1) * 512],
                    eps,
                    mybir.ActivationFunctionType.Tanh,
                    bias=rb[:, b : b + 1],
                    scale=1.0,
                )

        def energies(b, h):
            # per t-chunk matmul, t on partitions: [128, 4] half of [128, NT]
            if h == 0:
                expE_tiles[b] = (
                    psm.tile([128, NT], f32, tag="small", name=f"en{b}"),
                    sp.tile([128, NT], bf16, tag="exps", name=f"exps{b}"),
                    sp.tile([128, NT], bf16, tag="expE", name=f"expE{b}"),
                )
            enps, exps, expE = expE_tiles[b]
            for q in range(4):
                i = h * 4 + q
                nc.tensor.matmul(
                    enps[:, i : i + 1],
                    e_tiles[b][:, i * 128 : (i + 1) * 128],
                    w2s_b,
                    start=True,
                    stop=True,
                )
            # exp(relu(x + b2)) == max(exp(x + b2), 1)
            sl = slice(h * 4, (h + 1) * 4)
            nc.scalar.activation(
                exps[:, sl],
                enps[:, sl],
                mybir.ActivationFunctionType.Exp,
                bias=b2s[:, 0:1],
                scale=1.0,
            )
            nc.vector.tensor_scalar_max(expE[:, sl], exps[:, sl], 1.0)

        def context_half(b, h, cps):
            expE = expE_tiles[b][2]
            for q in range(4):
                i = h * 4 + q
                nc.tensor.matmul(
                    cps,
                    expE[:, i : i + 1],
                    xc_tiles[b][:, i, :],
                    start=(i == 0),
                    stop=(i == NT - 1),
                )

        def denominator(b):
            expE = expE_tiles[b][2]
            dps = psm.tile([1, NT], f32, tag="small", name=f"dps{b}")
            nc.tensor.matmul(dps, ones, expE, start=True, stop=True)
            den = sp.tile([1, 2], f32, tag="den", name=f"den{b}")
            nc.vector.reduce_sum(out=den[:, 0:1], in_=dps, axis=mybir.AxisListType.X)
            nc.vector.reciprocal(den[:, 1:2], den[:, 0:1])
            return den

        def context_finish(b, cps, den):
            ctx_sb = op.tile([1, D], f32, tag="ctx", name=f"ctx{b}")
            nc.vector.tensor_scalar_mul(ctx_sb, cps, den[:, 1:2])
            nc.sync.dma_start(out=out_d[b : b + 1, :], in_=ctx_sb)

        # ---- batch 0 head: transposes first (PE must not head-of-line
        # block on the weight chain), then the rnn/r computation ----
        transposes(0, halves=((0,), (1,)))
        w1a_b = consts.tile([128, ND, H], bf16)
        nc.vector.tensor_copy(w1a_b, w1a)
        w2s_b = consts.tile([H, 1], bf16)
        nc.vector.tensor_copy(w2s_b, w2s)
        e_mm(0)

        # ---- steady state with one-stage skew, split by T-halves ----
        cps_tiles = [None] * BP
        for b in range(BP):
            energies(b, 0)
            energies(b, 1)
            if b + 1 < BP:
                transposes(b + 1)
            den = denominator(b)
            cps_tiles[b] = pcx.tile([1, D], f32, tag="cps", name=f"cps{b}")
            context_half(b, 0, cps_tiles[b])
            context_half(b, 1, cps_tiles[b])
            context_finish(b, cps_tiles[b], den)
            if b + 1 < BP:
                e_mm(b + 1)

    nc.compile()
    return nc


def _get_nc():
    if "nc" not in _CACHE:
        _CACHE["nc"] = _build_nc()
    return _CACHE["nc"]


def _make_in_maps(cbhg, rnn, w1, b1, w2, b2):
    # rb = (rnn @ W1[D:] + b1).T per core: 20K FLOPs of input marshalling
    # (0.005% of the problem) done in float64 on the host
    w1b = np.asarray(w1[D:], dtype=np.float64)
    maps = []
    for c in range(NCORES):
        rnn_c = np.ascontiguousarray(rnn[c * BP : (c + 1) * BP])
        rb = (rnn_c.astype(np.float64) @ w1b + b1.astype(np.float64)).T
        maps.append(
            {
                "x": np.ascontiguousarray(cbhg[c * BP : (c + 1) * BP]),
                "rnn": rnn_c,
                "w1": w1,
                "b1": b1,
                "w2": w2,
                "b2": b2,
                "rb": np.ascontiguousarray(rb.astype(np.float32)),
            }
        )
    return maps


def _run(in_maps, trace=False):
    from concourse.bass_utils import run_bass_kernel_spmd

    nc = _get_nc()
    return run_bass_kernel_spmd(nc, in_maps, core_ids=list(range(NCORES)), trace=trace)


def kernel(cbhg_encoding, attention_rnn_output, W1, b1, W2, b2):
    cbhg = np.asarray(cbhg_encoding, dtype=np.float32)
    rnn = np.asarray(attention_rnn_output, dtype=np.float32)
    w1 = np.ascontiguousarray(np.asarray(W1, dtype=np.float32))
    b1v = np.ascontiguousarray(np.asarray(b1, dtype=np.float32))
    w2 = np.ascontiguousarray(np.asarray(W2, dtype=np.float32))
    b2v = np.ascontiguousarray(np.asarray(b2, dtype=np.float32))

    res = _run(_make_in_maps(cbhg, rnn, w1, b1v, w2, b2v))
    context = np.concatenate(
        [res.results[c]["ctx_out"][:, None, :] for c in range(NCORES)], axis=0
    ).astype(np.float32)
    rnn_reshaped = rnn.reshape(B, 1, D).copy()
    return (context, rnn_reshaped)

